# revision 66
# baseline (speedup 1.0000x reference)
"""BitNet attention (GQA, 32 q-heads / 8 kv-heads, hidden 4096, seq 2048) on 8
Trainium2 NeuronCores.

Sharding: tensor-parallel over heads. Core i computes q-heads 4i..4i+3 and
kv-head i (N_REP=4, so the 4 q-heads of core i attend exactly to kv-head i),
plus the o_proj contribution of its 512 hidden columns; the host sums the 8
partial o_proj outputs.

Device-side design (per core):
  - Q/K projections run in fp8 e4m3 with perf_mode=DoubleRow: x is
    quantized to e4m3 on the host and chunk PAIRS feed each matmul as the
    two DoubleRow subtiles, so a 256-deep contraction streams in one
    512-col pass -- measured 2.0x over bf16. The resulting q/k error is
    softmax-damped (measured ~1.2e-2 total vs the 2e-2 gate); V/AV/o_proj
    stay effectively full precision (fp8 there fails the gate: 3-6e-2).
  - x ships ONCE as fp8 pairs (x8, r8 = x - x8): the V projection uses
    DoubleRow with the SAME weight chunk in both subtiles over (x8, r8),
    reconstructing ~14-bit x at bf16-matmul cost, 2/3 the DMA bytes.
  - Scores stay bf16 (contraction = head_dim = 128, DoubleRow needs 256):
    S.T[k, q] = Kt_tile^T @ Qt; exp output P.T feeds the O.T and E matmuls
    with no transposes. Scores PSUM tiles span 2 banks so each ACT exp op
    amortizes its ~352-cycle overhead. No max-subtraction (|scores| ~ 1 by
    construction).
  - Phase 2 is ACT-exp-bound (~9.1us/body: 8 exps of (352+1024) cycles at
    1.2GHz). The PE's slack absorbs: E row-sums on DVE-prefolded tiles
    (folds run at body start, out-of-place, on prev's completed exps), and
    2 injected o_proj half-blocks per body (the freed psE 'sc' bank is the
    injection PSUM bank), shrinking phase 3.
  - E uses an all-ones 128x128 stationary so Sum_k P is produced already
    partition-broadcast; one DVE reciprocal then yields 1/E [128, 512]
    directly, and finalize is a single DVE multiply (sv is folded into the
    V.T copyout's per-partition ACT scale).
  - Iterations (h0,qb0) and (h1,qb0) have their scores+exp pre-emitted
    into the phase-1 tail so the exp pipeline starts two bodies ahead.
  - Beware: all engines near-saturated triggers the 50%-util power
    throttle (throttle_activity_1); run-to-run variance on shared devices
    is +/-20%. Packing more PE work per body than the ACT bound is a net
    loss through the throttle.
"""

import numpy as np
import ml_dtypes

import concourse.bass as bass
import concourse.mybir as mybir
import concourse.tile as tile
from concourse.vector_clock import ScopedClock
from concourse.bass_utils import run_bass_kernel_spmd

F32 = mybir.dt.float32
BF16 = mybir.dt.bfloat16
F8 = mybir.dt.float8e4

HIDDEN = 4096
T = 2048          # sequence length
N_CORES = 8
FQ = HIDDEN // N_CORES   # 512 q-features per core
H = 4                    # q heads per core
DH = 128                 # head dim
DC = HIDDEN // 128       # 32 contraction chunks
HC = DC // 2             # 16 chunks per xt half
TQ = 4                   # token quarters (512 tokens each)
KT = T // 128            # 16 key tiles
QB = 4                   # query blocks of 512

TRACE = False            # set by test.py for profiling runs
TRACE_ALL_CORES = False
INJECT_UNITS = True

_MAX_DRAIN_WAITS = 1
_MAX_INST_WAITS = 1


def _split_sync_waits(nc):
    """The walrus build in this container rejects instructions carrying more
    than one sync wait ("Too many sync wait commands"). Cap every instruction
    at _MAX_INST_WAITS waits; spill the excess onto InstEventSemaphore
    (standalone wait) instructions inserted immediately before on the same
    engine (engines are in-order, so combined wait semantics are identical)."""
    counter = [0]

    def _mk_wait(engine, waits):
        counter[0] += 1
        nop = mybir.InstEventSemaphore(
            name=f"waitsplit_{counter[0]}", ins=[], outs=[]
        )
        nop.engine = engine
        nop.sync_info = mybir.SyncInfo(on_wait=list(waits), on_update=[])
        nc.register_instruction(nop, overwrite=True)
        return nop

    for bb in nc.main_func.blocks:
        insts = list(bb.instructions)
        out = []
        changed = False
        for ins in insts:
            si = ins.sync_info
            waits = list(si.on_wait or []) if si else []
            if len(waits) > _MAX_INST_WAITS:
                changed = True
                rest = waits[:-_MAX_INST_WAITS]
                for i in range(0, len(rest), _MAX_INST_WAITS):
                    out.append(_mk_wait(ins.engine, rest[i : i + _MAX_INST_WAITS]))
                ins.sync_info = mybir.SyncInfo(
                    on_wait=waits[-_MAX_INST_WAITS:],
                    on_update=list(si.on_update or []),
                )
            out.append(ins)
        if changed:
            bb.instructions = out


class _PatchedTileContext(tile.TileContext):
    """Split the end-of-kernel drain's sem waits the same way (the drain is
    emitted after scheduling, outside _split_sync_waits' reach)."""

    def _drain_and_barrier(self, tick_clock, wait_clock):
        nc = self.nc
        drain_inst = nc.sync.drain()
        wait_clock.add_sem_waits(
            drain_inst.ins, ScopedClock({None: tick_clock.global_clock})
        )
        ins = drain_inst.ins
        si = ins.sync_info
        waits = list(si.on_wait or []) if si else []
        updates = list(si.on_update or []) if si else []
        if len(waits) > _MAX_DRAIN_WAITS:
            ins.sync_info = mybir.SyncInfo(
                on_wait=waits[:_MAX_DRAIN_WAITS], on_update=updates
            )
            rest = waits[_MAX_DRAIN_WAITS:]
            for i in range(0, len(rest), _MAX_DRAIN_WAITS):
                nop = nc.sync.nop(nofuse=True, hint=f"dw{i}")
                nop.ins.sync_info = mybir.SyncInfo(
                    on_wait=rest[i : i + _MAX_DRAIN_WAITS], on_update=[]
                )
        nc.all_engine_barrier()
        assert self.sems is not None
        popped = nc._tile_sem_poison_stack.pop()
        assert popped is self._sem_poison
        nc.clear_and_free_semaphores(list(self.sems.allocated().values()))
        nc.all_engine_barrier()


def _build(split_waits=True):
    nc = bass.Bass()

    # partition-major packed inputs (see _make_in_maps)
    # xq8[..., 0, :] = e4m3(x); [..., 1, :] = e4m3(x - e4m3(x)) residual
    xq8_d = nc.dram_tensor("xq8", [TQ, 2, 128, HC, 2, 512], F8,
                           kind="ExternalInput")
    bqt_d = nc.dram_tensor("bqt", [128, DC, FQ], F8, kind="ExternalInput")
    bkt_d = nc.dram_tensor("bkt", [128, DC, DH], F8, kind="ExternalInput")
    bvt_d = nc.dram_tensor("bvt", [128, DC, 2, DH], F8, kind="ExternalInput")
    bot_d = nc.dram_tensor("bot", [4, 128, H, 1024], BF16, kind="ExternalInput")
    sq_d = nc.dram_tensor("sq", [H, DH, 1], F32, kind="ExternalInput")
    sk_d = nc.dram_tensor("sk", [DH, 1], F32, kind="ExternalInput")
    sv_d = nc.dram_tensor("sv", [DH, 1], F32, kind="ExternalInput")
    onesm_d = nc.dram_tensor("onesm", [128, 128], BF16, kind="ExternalInput")
    ident_d = nc.dram_tensor("ident", [128, 128], BF16, kind="ExternalInput")
    y_d = nc.dram_tensor("y", [T, HIDDEN], BF16, kind="ExternalOutput")

    from contextlib import ExitStack
    with _PatchedTileContext(nc) as tc, ExitStack() as _ctx:
        wq = _ctx.enter_context(tc.tile_pool(name="wq", bufs=1))
        wk = _ctx.enter_context(tc.tile_pool(name="wk", bufs=1))
        wv = _ctx.enter_context(tc.tile_pool(name="wv", bufs=1))
        xq8p = _ctx.enter_context(tc.tile_pool(name="xq8", bufs=3))
        qtp = _ctx.enter_context(tc.tile_pool(name="qt", bufs=H))
        ktp = _ctx.enter_context(tc.tile_pool(name="kt", bufs=1))
        vvp = _ctx.enter_context(tc.tile_pool(name="vv", bufs=TQ))
        ptp = _ctx.enter_context(tc.tile_pool(name="pt", bufs=16))
        foldp = _ctx.enter_context(tc.tile_pool(name="fold", bufs=2))
        otp = _ctx.enter_context(tc.tile_pool(name="ot", bufs=H))
        wop = _ctx.enter_context(tc.tile_pool(name="wo", bufs=4))
        ysp = _ctx.enter_context(tc.tile_pool(name="ys", bufs=3))
        vtp = _ctx.enter_context(tc.tile_pool(name="vt", bufs=2))
        scp = _ctx.enter_context(tc.tile_pool(name="sc", bufs=2))
        misc = _ctx.enter_context(tc.tile_pool(name="misc", bufs=2))
        psM = _ctx.enter_context(tc.tile_pool(name="psM", bufs=2, space="PSUM"))
        psS = _ctx.enter_context(tc.tile_pool(name="psS", bufs=2, space="PSUM"))
        psE = _ctx.enter_context(tc.tile_pool(name="psE", bufs=1, space="PSUM"))
        psY = _ctx.enter_context(tc.tile_pool(name="psY", bufs=1, space="PSUM"))
        if True:
            # --- xq8 first pieces of tq0 first so compute starts ASAP ----
            xq8_sb = {}  # (tq, half) -> [128, HC, 2, 512] fp8 tile

            def load_xt(tq, half):
                t8 = xq8p.tile([128, HC, 2, 512], F8, tag="xq8",
                               name=f"xq8_{tq}_{half}")
                nc.sync.dma_start(t8[:, : HC // 2], xq8_d[tq, half, :, : HC // 2])
                nc.sync.dma_start(t8[:, HC // 2 :], xq8_d[tq, half, :, HC // 2 :])
                xq8_sb[(tq, half)] = t8

            def xt8_pair(tq, c2):
                # x8 of chunk pair (2c, 2c+1) as a DoubleRow rhs [128, 2, 512]
                half, m = divmod(2 * c2, HC)
                return xq8_sb[(tq, half)][:, m : m + 2, 0, :]

            def xr8_chunk(tq, dc):
                # (x8, r8) of chunk dc as a DoubleRow rhs [128, 2, 512]
                half, m = divmod(dc, HC)
                return xq8_sb[(tq, half)][:, m, :, :]

            bqt_sb = wq.tile([128, DC, FQ], F8, tag="wq")
            bkt_sb = wk.tile([128, DC, DH], F8, tag="wk")
            bvt_sb = wv.tile([128, DC, 2, DH], F8, tag="wv")
            # fine-grained interleave so the first Q matmuls (need bqt
            # chunk dc + xq8 chunk dc) can start as early as possible
            Q4 = DC // 8  # 4 chunks per piece
            for piece in range(8):
                csl = slice(piece * Q4, (piece + 1) * Q4)
                nc.sync.dma_start(bqt_sb[:, csl], bqt_d[:, csl])
                half_ = piece // 4
                hsl = slice((piece % 4) * Q4, (piece % 4) * Q4 + Q4)
                if piece % 4 == 0:
                    xq8_sb[(0, half_)] = xq8p.tile(
                        [128, HC, 2, 512], F8, tag="xq8", name=f"xq8_0_{half_}"
                    )
                nc.sync.dma_start(
                    xq8_sb[(0, half_)][:, hsl], xq8_d[0, half_, :, hsl]
                )
            nc.sync.dma_start(bkt_sb[:], bkt_d[:])
            nc.sync.dma_start(bvt_sb[:, :HC], bvt_d[:, :HC])
            nc.sync.dma_start(bvt_sb[:, HC:], bvt_d[:, HC:])

            # --- constants / scales -------------------------------------
            sq_sb = [misc.tile([DH, 1], F32, tag=f"sq{f}", name=f"sq{f}")
                     for f in range(H)]
            for f in range(H):
                nc.sync.dma_start(sq_sb[f][:], sq_d[f])
            sk_sb = misc.tile([DH, 1], F32, tag="sk")
            nc.sync.dma_start(sk_sb[:], sk_d[:])
            sv_sb = misc.tile([DH, 1], F32, tag="sv")
            nc.sync.dma_start(sv_sb[:], sv_d[:])
            onesm_sb = misc.tile([128, 128], BF16, tag="onesm")
            nc.sync.dma_start(onesm_sb[:], onesm_d[:])
            ident_sb = misc.tile([128, 128], BF16, tag="ident")
            nc.sync.dma_start(ident_sb[:], ident_d[:])

            # --- persistent activation tiles ----------------------------
            qt_sb = [qtp.tile([DH, T], BF16, tag="qt", name=f"qt{f}")
                     for f in range(H)]
            kt_sb = ktp.tile([DH, T], BF16, tag="kt")
            vv_sb = [vvp.tile([128, 512], BF16, tag="vv", name=f"vv{tq}")
                     for tq in range(TQ)]
            ot_sb = [otp.tile([DH, T], BF16, tag="ot", name=f"ot{f}")
                     for f in range(H)]

            # --- phase 1: q/k/v projections, one token-quarter at a time
            def emit_q(tq, f):
                tsl = slice(tq * 512, (tq + 1) * 512)
                ps = psM.tile([128, 512], F32, tag="mm", name=f"psq{tq}_{f}")
                for c2 in range(DC // 2):
                    nc.tensor.matmul(
                        ps[:],
                        bqt_sb[:, 2 * c2 : 2 * c2 + 2,
                               f * 128 : (f + 1) * 128],
                        xt8_pair(tq, c2),
                        start=(c2 == 0), stop=(c2 == DC // 2 - 1),
                        perf_mode=mybir.MatmulPerfMode.DoubleRow,
                    )
                nc.scalar.activation(
                    qt_sb[f][:, tsl], ps[:],
                    mybir.ActivationFunctionType.Copy, scale=sq_sb[f][:],
                )

            def emit_k(tq):
                tsl = slice(tq * 512, (tq + 1) * 512)
                ps = psM.tile([128, 512], F32, tag="mm", name=f"psk{tq}")
                for c2 in range(DC // 2):
                    nc.tensor.matmul(
                        ps[:], bkt_sb[:, 2 * c2 : 2 * c2 + 2, :],
                        xt8_pair(tq, c2),
                        start=(c2 == 0), stop=(c2 == DC // 2 - 1),
                        perf_mode=mybir.MatmulPerfMode.DoubleRow,
                    )
                nc.scalar.activation(
                    kt_sb[:, tsl], ps[:],
                    mybir.ActivationFunctionType.Copy, scale=sk_sb[:],
                )

            def emit_v(tq):
                # V.T = bv^T(x8 + r8): DoubleRow with the SAME weight chunk in
                # both subtiles and (x8, r8) as the moving subtiles -- full
                # x precision at one matmul per chunk. Then 4 PE transposes
                # back to [t, d].
                ps = psM.tile([128, 512], F32, tag="mm", name=f"psv{tq}")
                for dc in range(DC):
                    nc.tensor.matmul(
                        ps[:], bvt_sb[:, dc], xr8_chunk(tq, dc),
                        start=(dc == 0), stop=(dc == DC - 1),
                        perf_mode=mybir.MatmulPerfMode.DoubleRow,
                    )
                # sv folds into the V.T copyout (per-partition ACT scale), so
                # the attention output needs only the 1/E multiply
                vt_sb = vtp.tile([128, 512], BF16, tag="vt", name=f"vt{tq}")
                nc.scalar.activation(
                    vt_sb[:], ps[:],
                    mybir.ActivationFunctionType.Copy, scale=sv_sb[:],
                )
                for vt in range(4):
                    ps_tr = psS.tile([128, 128], BF16, tag="s2",
                                     name=f"pstr{tq}_{vt}")
                    nc.tensor.transpose(
                        ps_tr[:], vt_sb[:, vt * 128 : (vt + 1) * 128],
                        ident_sb[:],
                    )
                    nc.vector.tensor_copy(
                        out=vv_sb[tq][:, vt * 128 : (vt + 1) * 128],
                        in_=ps_tr[:],
                    )

            def emit_score_pair(h, qb, kp, pt_list):
                qsl = slice(qb * 512, (qb + 1) * 512)
                ps_s = psS.tile([128, 1024], F32, tag="s2",
                                name=f"pss{h}_{qb}_{kp}")
                for j in range(2):
                    kt = 2 * kp + j
                    nc.tensor.matmul(
                        ps_s[:, j * 512 : (j + 1) * 512],
                        kt_sb[:, kt * 128 : (kt + 1) * 128],
                        qt_sb[h][:, qsl],
                        start=True, stop=True,
                    )
                pt = ptp.tile([128, 1024], BF16, tag="pt",
                              name=f"pt{h}_{qb}_{kp}")
                nc.scalar.activation(
                    pt[:], ps_s[:], mybir.ActivationFunctionType.Exp
                )
                pt_list.append(pt)

            pro_pt = []   # iteration (h0, qb0) scores, emitted into tq3
            pro_pt2 = []  # iteration (h1, qb0) scores, also pre-emitted so
                          # the ACT exp pipeline starts two bodies ahead
            for tq in range(TQ):
                if tq > 0:
                    load_xt(tq, 0)
                    load_xt(tq, 1)
                if tq < 3:
                    for f in range(H):
                        emit_q(tq, f)
                    emit_k(tq)
                    emit_v(tq)
                else:
                    # interleave iteration-0/1 scores into the projection
                    # tail: Kt completes after emit_k(3); Qt[h] after
                    # emit_q(3, h)
                    emit_q(3, 0)
                    emit_k(3)
                    emit_q(3, 1)
                    for kp in range(0, 4):
                        emit_score_pair(0, 0, kp, pro_pt)
                    emit_q(3, 2)
                    for kp in range(4, 8):
                        emit_score_pair(0, 0, kp, pro_pt)
                    for kp in range(0, 3):
                        emit_score_pair(1, 0, kp, pro_pt2)
                    emit_q(3, 3)
                    emit_v(3)
                    for kp in range(3, 8):
                        emit_score_pair(1, 0, kp, pro_pt2)

            # --- prefetch all o_proj weight blocks: the DMA queue is idle
            # during phase 2, so these 4MB land long before phase 3 --------
            bot_sb = []
            for obp in range(4):
                t_ = wop.tile([128, H, 1024], BF16, tag="wo", name=f"wo{obp}")
                nc.sync.dma_start(t_[:], bot_d[obp])
                bot_sb.append(t_)

            # --- o_proj half-block units: (obp, tt, jo). Ordered qb-major so
            # phase 2 can inject the earliest units (their ot slices finalize
            # first) into the PE's ACT-exp wait slack; phase 3 does the rest.
            oproj_units = [
                (obp, tt, jo)
                for qb in range(QB)
                for tt in range(qb * 4, qb * 4 + 4)
                for obp in range(4)
                for jo in range(2)
            ]
            o_next = [0]

            def emit_oproj_unit(inject=False):
                obp, tt, jo = oproj_units[o_next[0]]
                o_next[0] += 1
                ps_y = psY.tile([128, 512], F32, tag="y",
                                name=f"psyi{obp}_{tt}_{jo}")
                for c in range(H):
                    nc.tensor.matmul(
                        ps_y[:],
                        ot_sb[c][:, tt * 128 : (tt + 1) * 128],
                        bot_sb[obp][:, c, jo * 512 : (jo + 1) * 512],
                        start=(c == 0), stop=(c == H - 1),
                    )
                ysb = ysp.tile([128, 1024], BF16, tag="ys",
                               name=f"ysi{obp}_{tt}_{jo}")
                nc.vector.tensor_copy(out=ysb[:, :512], in_=ps_y[:])
                nc.sync.dma_start(
                    y_d[tt * 128 : (tt + 1) * 128,
                        obp * 1024 + jo * 512 : obp * 1024 + (jo + 1) * 512],
                    ysb[:, :512],
                )

            # --- phase 2: attention, software-pipelined ------------------
            # Iteration J = (h, qb). Body(idx) interleaves, at score-pair
            # granularity: scores+exp of iters[idx] with the O.T/E matmuls of
            # iters[idx-1]; the softmax-denominator chain of iters[idx-1]
            # (E copyout -> reciprocal on DVE) is issued at body end, and its
            # PE-side finalize (sv (x) 1/E outer product + copy + multiply)
            # lands mid-body(idx+1), by which time the reciprocal is done --
            # the PE never waits on the DVE.
            iters = [(h, qb) for qb in range(QB) for h in range(H)]
            pend_fin = []
            prev = (0, 0, pro_pt, None, None, None)

            def finalize2(st):
                # sv is already folded into V; the E row-sum used an all-ones
                # 128x128 stationary so 1/E is already partition-broadcast
                h, qb, _, ps_o, _, recip_bc = st
                qsl = slice(qb * 512, (qb + 1) * 512)
                nc.vector.tensor_tensor(
                    ot_sb[h][:, qsl], ps_o[:], recip_bc[:], mybir.AluOpType.mult
                )

            for idx in range(1, len(iters) + 1):
                cur = iters[idx] if idx < len(iters) else None
                new_pt = []
                f_lo = f_hi = None
                if prev is not None:
                    ph, pqb, ppt, _, _, _ = prev
                    ps_o = psM.tile([128, 512], F32, tag="mm",
                                    name=f"pso{ph}_{pqb}")
                    ps_e = psE.tile([128, 512], F32, tag="e",
                                    name=f"pse{ph}_{pqb}")
                    prev = (ph, pqb, ppt, ps_o, ps_e, None)
                    # out-of-place row-sum folds at body start: prev's exp
                    # tiles are all complete, so the DVE can run these 6 adds
                    # while the PE streams scores -- the E matmuls mid-body
                    # then never wait on the DVE.
                    f_lo = foldp.tile([128, 1024], BF16, tag="f",
                                      name=f"flo{ph}_{pqb}")
                    f_hi = foldp.tile([128, 1024], BF16, tag="f",
                                      name=f"fhi{ph}_{pqb}")
                    nc.vector.tensor_tensor(
                        f_lo[:], ppt[0][:], ppt[1][:], mybir.AluOpType.add)
                    nc.vector.tensor_tensor(
                        f_lo[:], f_lo[:], ppt[2][:], mybir.AluOpType.add)
                    nc.vector.tensor_tensor(
                        f_lo[:], f_lo[:], ppt[3][:], mybir.AluOpType.add)
                for kpp in range(KT // 4):
                    for kp in (2 * kpp, 2 * kpp + 1):
                        if cur is not None and idx >= 2:
                            h, qb = cur
                            qsl = slice(qb * 512, (qb + 1) * 512)
                            ps_s = psS.tile([128, 1024], F32, tag="s2",
                                            name=f"pss{h}_{qb}_{kp}")
                            for j in range(2):
                                kt = 2 * kp + j
                                nc.tensor.matmul(
                                    ps_s[:, j * 512 : (j + 1) * 512],
                                    kt_sb[:, kt * 128 : (kt + 1) * 128],
                                    qt_sb[h][:, qsl],
                                    start=True, stop=True,
                                )
                            pt = ptp.tile([128, 1024], BF16, tag="pt",
                                          name=f"pt{h}_{qb}_{kp}")
                            nc.scalar.activation(
                                pt[:], ps_s[:],
                                mybir.ActivationFunctionType.Exp,
                            )
                            new_pt.append(pt)
                    if prev is not None:
                        ph, pqb, ppt, ps_o, ps_e, _ = prev
                        for kt in range(4 * kpp, 4 * kpp + 4):
                            tqi, vti = divmod(kt, 4)
                            rhs = ppt[kt // 2][:, (kt % 2) * 512
                                               : (kt % 2 + 1) * 512]
                            nc.tensor.matmul(
                                ps_o[:],
                                vv_sb[tqi][:, vti * 128 : (vti + 1) * 128],
                                rhs,
                                start=(kt == 0), stop=(kt == KT - 1),
                            )
                    if kpp == 2 and pend_fin:
                        finalize2(pend_fin.pop(0))
                    if INJECT_UNITS and kpp in (1, 3) and o_next[0] < len(oproj_units):
                        # use PE slack (phase 2 is ACT-exp-bound) for o_proj
                        # units whose ot slices are already finalized; placed
                        # before this body's finalize so the tile-level ot_sb
                        # write-read ordering never makes the PE wait on it
                        nq = oproj_units[o_next[0]][1] // 4
                        if nq * 4 + 5 < idx:
                            emit_oproj_unit(inject=True)
                    if kpp == 2 and prev is not None:
                        # f_hi folds wait on prev's LAST exps (ACT runs one
                        # body behind), so they're emitted after the kpp1
                        # injection's DVE copyout -- otherwise that copyout
                        # queues behind them and the kpp3 injection stalls on
                        # the psY bank (bufs=1)
                        ph, pqb, ppt, ps_o, ps_e, _ = prev
                        nc.vector.tensor_tensor(
                            f_hi[:], ppt[4][:], ppt[5][:], mybir.AluOpType.add)
                        nc.vector.tensor_tensor(
                            f_hi[:], f_hi[:], ppt[6][:], mybir.AluOpType.add)
                        nc.vector.tensor_tensor(
                            f_hi[:], f_hi[:], ppt[7][:], mybir.AluOpType.add)
                if prev is not None:
                    ph, pqb, ppt, ps_o, ps_e, _ = prev
                    nc.vector.tensor_tensor(
                        f_lo[:], f_lo[:], f_hi[:], mybir.AluOpType.add)
                    for j in range(2):
                        nc.tensor.matmul(
                            ps_e[:], onesm_sb[:],
                            f_lo[:, j * 512 : (j + 1) * 512],
                            start=(j == 0), stop=(j == 1),
                        )
                    recip_bc = scp.tile([128, 512], F32, tag="rbc",
                                        name=f"rbc{ph}_{pqb}")
                    nc.vector.reciprocal(recip_bc[:], ps_e[:])
                    pend_fin.append((ph, pqb, ppt, ps_o, ps_e, recip_bc))
                if cur is None:
                    prev = None
                elif idx == 1:
                    prev = (cur[0], cur[1], pro_pt2, None, None, None)
                else:
                    prev = (cur[0], cur[1], new_pt, None, None, None)

            # --- phase 3: o_proj remainder ------------------------------
            # Consume the units phase 2 didn't inject. Full (obp, tt) pairs
            # share one 2-bank PSUM tile from the (now idle) scores pool so
            # each DVE copyout amortizes its overhead; a lone jo half at the
            # injection boundary goes through the psY bank.
            emitted = 0
            while o_next[0] < len(oproj_units):
                u0 = oproj_units[o_next[0]]
                pair = (
                    o_next[0] + 1 < len(oproj_units)
                    and oproj_units[o_next[0] + 1][:2] == u0[:2]
                    and u0[2] == 0
                )
                if pair:
                    obp, tt, _ = u0
                    o_next[0] += 2
                    ps_y = psS.tile([128, 1024], F32, tag="s2",
                                    name=f"psy{obp}_{tt}")
                    for jo in range(2):
                        for c in range(H):
                            nc.tensor.matmul(
                                ps_y[:, jo * 512 : (jo + 1) * 512],
                                ot_sb[c][:, tt * 128 : (tt + 1) * 128],
                                bot_sb[obp][:, c, jo * 512 : (jo + 1) * 512],
                                start=(c == 0), stop=(c == H - 1),
                            )
                    ysb = ysp.tile([128, 1024], BF16, tag="ys",
                                   name=f"ys{obp}_{tt}")
                    nc.vector.tensor_copy(out=ysb[:], in_=ps_y[:])
                    nc.sync.dma_start(
                        y_d[tt * 128 : (tt + 1) * 128,
                            obp * 1024 : (obp + 1) * 1024],
                        ysb[:],
                    )
                else:
                    emit_oproj_unit()
                emitted += 1
                if emitted == 6:
                    while pend_fin:
                        finalize2(pend_fin.pop(0))

    if split_waits:
        _split_sync_waits(nc)
    return nc


_NC_CACHE = None


def _get_nc():
    global _NC_CACHE
    if _NC_CACHE is None:
        _NC_CACHE = _build()
    return _NC_CACHE


def _binarize(w):
    """Match reference bitnet_linear: s = max(mean|W|_row, 1e-8) (>0), so
    sign(W/s) == sign(W). Returns (sign(W) as bf16, s as f32)."""
    w = np.asarray(w, np.float32)
    s = np.maximum(
        np.abs(w).mean(axis=1, dtype=np.float64).astype(np.float32), 1e-8
    )
    return np.sign(w).astype(ml_dtypes.bfloat16), s


def _make_in_maps(hidden_states, q_weight, q_scale, k_weight, k_scale,
                  v_weight, v_scale, o_weight, o_scale):
    hs = np.asarray(hidden_states, np.float32)
    b, t, hid = hs.shape
    assert (b, t, hid) == (1, T, HIDDEN)

    xT = np.ascontiguousarray(hs[0].T)
    # x = x8 + r8 (+ eps): e4m3 main + e4m3 residual, packed together as
    # [tq, half, p, c_in_half, 2, f]   (d = (half*HC + c)*128 + p,
    #                                   t = tq*512 + f)
    x8 = xT.astype(ml_dtypes.float8_e4m3)
    r8 = (xT - x8.astype(np.float32)).astype(ml_dtypes.float8_e4m3)
    xq8 = np.ascontiguousarray(
        np.stack([x8, r8], axis=-2)           # [d, 2, t]
        .reshape(2, HC, 128, 2, TQ, 512)
        .transpose(4, 0, 2, 1, 3, 5)
    )

    bq, s_q = _binarize(q_weight)
    bk, s_k = _binarize(k_weight)
    bv, s_v = _binarize(v_weight)
    bo, s_o = _binarize(o_weight)

    sq_full = s_q * np.asarray(q_scale, np.float32)                # [4096]
    sk_full = s_k * np.asarray(k_scale, np.float32) / np.sqrt(DH)  # [1024]
    sv_full = s_v * np.asarray(v_scale, np.float32)                # [1024]
    so_full = s_o * np.asarray(o_scale, np.float32)                # [4096]

    onesm = np.ones((128, 128), ml_dtypes.bfloat16)
    ident = np.eye(128, dtype=ml_dtypes.bfloat16)

    def pack_w(wt, nf):
        # [d, nf] -> [p, c, nf]
        return np.ascontiguousarray(wt.reshape(DC, 128, nf).transpose(1, 0, 2))

    in_maps = []
    for i in range(N_CORES):
        fq = slice(FQ * i, FQ * (i + 1))
        fk = slice(DH * i, DH * (i + 1))
        bot = np.ascontiguousarray(bo[:, fq].T)  # [512 cfeat, 4096 o]
        bvt8 = pack_w(np.ascontiguousarray(bv[fk].T), DH).astype(
            ml_dtypes.float8_e4m3
        )
        in_maps.append({
            "xq8": xq8,
            "bqt": pack_w(np.ascontiguousarray(bq[fq].T), FQ).astype(
                ml_dtypes.float8_e4m3
            ),
            "bkt": pack_w(np.ascontiguousarray(bk[fk].T), DH).astype(
                ml_dtypes.float8_e4m3
            ),
            "bvt": np.ascontiguousarray(
                np.repeat(bvt8[:, :, None, :], 2, axis=2)
            ),
            "bot": np.ascontiguousarray(
                bot.reshape(H, 128, 4, 1024).transpose(2, 1, 0, 3)
            ),
            "sq": np.ascontiguousarray(
                sq_full[fq].reshape(H, DH, 1).astype(np.float32)
            ),
            "sk": np.ascontiguousarray(
                sk_full[fk].reshape(DH, 1).astype(np.float32)
            ),
            "sv": np.ascontiguousarray(
                sv_full[fk].reshape(DH, 1).astype(np.float32)
            ),
            "onesm": onesm,
            "ident": ident,
        })
    return in_maps, so_full


def kernel(**inputs):
    in_maps, so_full = _make_in_maps(**inputs)
    nc = _get_nc()
    res = run_bass_kernel_spmd(
        nc, in_maps, core_ids=list(range(N_CORES)), trace=TRACE,
        trace_cores=list(range(N_CORES)) if TRACE and TRACE_ALL_CORES else None,
    )
    if TRACE:
        kernel.last_exec_time_ns = res.exec_time_ns
        kernel.last_mean_exec_time_ns = res.mean_exec_time_ns

    y = np.zeros((T, HIDDEN), np.float32)
    for i in range(N_CORES):
        y += res.results[i]["y"].astype(np.float32)
    y *= so_full[None, :]
    return y.reshape(1, T, HIDDEN)

